# revision 7
# baseline (speedup 1.0000x reference)
"""Trainium2 Bass kernel for nn_AnemllQATLinearRefactored.

Reference math:
    A  = tanh(scale_A)                  [O, R]
    Bm = tanh(scale_B)                  [R, I]
    Ag = A * rank_magnitude             [O, R]
    y  = einsum("bsi,ri,oi,or->bso", x, Bm, Q, Ag) + bias

Refactored (exact):
    S2   = Ag @ Bm                      [O, I]
    Weff = Q * S2                       [O, I]   (elementwise)
    y    = x @ Weff.T + bias            -> ONE big matmul instead of R

Distribution: column-parallel over out_features across 8 NeuronCores
(shard weight/bias/scale_A along O; replicate x/scale_B/rank_magnitude).
Each core computes out[o_shard, n] = Weff_shard @ x.T + bias_shard; the
host concatenates the 8 output shards and transposes.

On-device per core:
  - Bg = tanh(scale_B) * rm            [4, 4096]   (ACT tanh + DVE mul)
  - A_t = tanh(scale_A_shard.T)        [4, 512]
  - per i-tile kt: S2T[i128, o512] = matmul(lhsT=Bg[:, kt], rhs=A_t)
    WeffT[:, kt, :] = S2T * wT_tile    (DVE)  -> resident SBUF [128,32,512]
  - main: for nt(8) x kt(32) x ot(4):
      psum[ot] += WeffT[kt, ot*128:+128].T @ xT[kt, nt*512:+512]
    (float32r matmuls: full-rate fp32-storage mode on the PE)
  - psum eviction adds bias (DVE) and DMAs out[o128, n512] tiles.
"""
import sys

if "/opt/trn_rl_repo" not in sys.path:
    sys.path.insert(0, "/opt/trn_rl_repo")

import numpy as np

import concourse.bass as bass  # noqa: F401  (engine types)
import concourse.mybir as mybir
import concourse.tile as tile
from concourse import bacc
from concourse.bass_utils import run_bass_kernel_spmd

# Problem shapes (hardcoded per harness contract)
B, S, I, O, R = 2, 2048, 4096, 4096, 4
N = B * S                  # 4096 rows
NCORES = 8
O_SH = O // NCORES         # 512 out-features per core
P = 128
KT = I // P                # 32 contraction tiles
NT = N // 512              # 8 moving-dim tiles of 512
OT = O_SH // P             # 4 psum partition tiles
KTG = 8                    # kt tiles per x DMA chunk
F32 = mybir.dt.float32
F32R = mybir.dt.float32r


def build_graph():
    nc = bacc.Bacc("TRN2", target_bir_lowering=False, debug=False)

    xT = nc.declare_dram_parameter("xT", [I, N], F32R, isOutput=False)
    wT = nc.declare_dram_parameter("wT", [I, O_SH], F32, isOutput=False)
    scale_a_t = nc.declare_dram_parameter("scale_a_t", [R, O_SH], F32R, isOutput=False)
    scale_b = nc.declare_dram_parameter("scale_b", [R, I], F32R, isOutput=False)
    rm = nc.declare_dram_parameter("rm", [R, 1], F32, isOutput=False)
    bias_t = nc.declare_dram_parameter("bias_t", [P, OT], F32, isOutput=False)
    out = nc.declare_dram_parameter("out", [O_SH, N], F32, isOutput=True)

    xT_r = xT.ap().rearrange("(kt p) n -> p kt n", p=P)     # [128, 32, 4096]
    out_r = out.ap().rearrange("(ot p) n -> p ot n", p=P)   # [128, 4, 4096]
    wT_r = wT.ap().rearrange("(kt p) o -> p kt o", p=P)     # [128, 32, 512]

    with tile.TileContext(nc) as tc:
        with tc.tile_pool(name="const", bufs=1) as const, \
             tc.tile_pool(name="wpool", bufs=3) as wpool, \
             tc.tile_pool(name="xpool", bufs=4) as xpool, \
             tc.tile_pool(name="opool", bufs=2) as opool, \
             tc.tile_pool(name="ps_out", bufs=5, space="PSUM") as ps_out_pool, \
             tc.tile_pool(name="ps_s2t", bufs=3, space="PSUM") as ps_s2t_pool:

            # ---- tiny constants ----
            sb_bg = const.tile([R, I], F32R)
            nc.sync.dma_start(sb_bg[:], scale_b.ap())
            nc.scalar.activation(sb_bg[:], sb_bg[:], mybir.ActivationFunctionType.Tanh)
            sb_rm = const.tile([R, 1], F32)
            nc.sync.dma_start(sb_rm[:], rm.ap())

            sb_at = const.tile([R, O_SH], F32R)
            nc.sync.dma_start(sb_at[:], scale_a_t.ap())
            nc.scalar.activation(sb_at[:], sb_at[:], mybir.ActivationFunctionType.Tanh)
            nc.vector.tensor_mul(sb_at[:], sb_at[:], sb_rm[:].to_broadcast((R, O_SH)))

            sb_bias = const.tile([P, OT], F32)
            nc.sync.dma_start(sb_bias[:], bias_t.ap())

            # ---- WeffT = wT * (Bg.T @ (A_t*rm)) built lazily, interleaved
            # into nt0's matmul stream with a small lookahead so the PE never
            # sits idle behind the DVE multiplies.
            weffT = const.tile([P, KT, O_SH], F32R)         # 64 KiB/partition
            WKTG = 4
            wt_tiles = {}

            def emit_wefft(kt):
                ktg, ktl = divmod(kt, WKTG)
                if ktl == 0:
                    wt = wpool.tile([P, WKTG, O_SH], F32, name=f"wt_{ktg}", tag="wt")
                    nc.sync.dma_start(wt[:], wT_r[:, ktg * WKTG:(ktg + 1) * WKTG, :])
                    wt_tiles[ktg] = wt
                ps = ps_s2t_pool.tile([P, O_SH], F32, name=f"ps_s2t_{kt}", tag="ps_s2t")
                nc.tensor.matmul(
                    ps[:],
                    sb_bg[:, kt * P:(kt + 1) * P],
                    sb_at[:],
                    start=True, stop=True,
                )
                nc.vector.tensor_mul(weffT[:, kt, :], ps[:], wt_tiles[ktg][:, ktl, :])

            LOOKAHEAD = 3
            for kt in range(LOOKAHEAD):
                emit_wefft(kt)

            # ---- main matmul: out[o, n] = WeffT.T @ xT + bias ----
            for nt in range(NT):
                x_tiles = []
                for ktg in range(KT // KTG):
                    xt = xpool.tile([P, KTG, 512], F32R, name=f"xt_{nt}_{ktg}", tag="xt")
                    nc.gpsimd.dma_start(
                        xt[:],
                        xT_r[:, ktg * KTG:(ktg + 1) * KTG, nt * 512:(nt + 1) * 512],
                    )
                    x_tiles.append(xt)
                ps_tiles = [
                    ps_out_pool.tile([P, 512], F32, name=f"ps_out_{nt}_{ot}", tag="ps_out")
                    for ot in range(OT)
                ]
                for kt in range(KT):
                    xt = x_tiles[kt // KTG]
                    for ot in range(OT):
                        nc.tensor.matmul(
                            ps_tiles[ot][:],
                            weffT[:, kt, ot * P:(ot + 1) * P],
                            xt[:, kt % KTG, :],
                            start=(kt == 0), stop=(kt == KT - 1),
                        )
                    if nt == 0 and kt + LOOKAHEAD < KT:
                        emit_wefft(kt + LOOKAHEAD)
                o_sb = opool.tile([P, OT, 512], F32, name=f"o_sb_{nt}", tag="o_sb")
                for ot in range(OT):
                    nc.vector.tensor_tensor(
                        o_sb[:, ot, :], ps_tiles[ot][:],
                        sb_bias[:, ot:ot + 1].to_broadcast((P, 512)),
                        mybir.AluOpType.add,
                    )
                nc.scalar.dma_start(
                    out_r[:, :, nt * 512:(nt + 1) * 512],
                    o_sb[:],
                )
    nc.compile()
    return nc


_NC_CACHE = None


def _get_graph():
    global _NC_CACHE
    if _NC_CACHE is None:
        _NC_CACHE = build_graph()
    return _NC_CACHE


def make_in_maps(x, weight, bias, scale_A, scale_B, rank_magnitude):
    x = np.asarray(x, dtype=np.float32)
    weight = np.asarray(weight, dtype=np.float32)
    bias = np.asarray(bias, dtype=np.float32)
    scale_A = np.asarray(scale_A, dtype=np.float32)
    scale_B = np.asarray(scale_B, dtype=np.float32)
    rank_magnitude = np.asarray(rank_magnitude, dtype=np.float32)

    xT = np.ascontiguousarray(x.reshape(N, I).T)            # [I, N]
    in_maps = []
    for c in range(NCORES):
        sl = slice(c * O_SH, (c + 1) * O_SH)
        in_maps.append({
            "xT": xT,
            "wT": np.ascontiguousarray(weight[sl].T),       # [I, O_SH]
            "scale_a_t": np.ascontiguousarray(scale_A[sl].T),  # [R, O_SH]
            "scale_b": scale_B,                              # [R, I]
            "rm": rank_magnitude.reshape(R, 1),
            "bias_t": np.ascontiguousarray(bias[sl].reshape(OT, P).T),  # [128, 4]
        })
    return in_maps


def run(in_maps, trace=False):
    nc = _get_graph()
    return run_bass_kernel_spmd(nc, in_maps, core_ids=list(range(NCORES)), trace=trace)


def assemble(results):
    parts = [results[c]["out"] for c in range(NCORES)]       # each [O_SH, N]
    y_t = np.concatenate(parts, axis=0)                      # [O, N]
    return np.ascontiguousarray(y_t.T).reshape(B, S, O)


def kernel(x, weight, bias, scale_A, scale_B, rank_magnitude):
    in_maps = make_in_maps(x, weight, bias, scale_A, scale_B, rank_magnitude)
    res = run(in_maps, trace=False)
    return assemble(res.results)


# revision 8
# speedup vs baseline: 1.3151x; 1.3151x over previous
"""Trainium2 Bass kernel for nn_AnemllQATLinearRefactored.

Reference math:
    A  = tanh(scale_A)                  [O, R]
    Bm = tanh(scale_B)                  [R, I]
    Ag = A * rank_magnitude             [O, R]
    y  = einsum("bsi,ri,oi,or->bso", x, Bm, Q, Ag) + bias

Refactored (exact):
    S2   = Ag @ Bm                      [O, I]
    Weff = Q * S2                       [O, I]   (elementwise)
    y    = x @ Weff.T + bias            -> ONE big matmul instead of R

Distribution: column-parallel over out_features across 8 NeuronCores
(shard weight/bias/scale_A along O; replicate x/scale_B/rank_magnitude).
Each core computes out[o_shard, n] = Weff_shard @ x.T + bias_shard; the
host concatenates the 8 output shards and transposes.

On-device per core:
  - Bg = tanh(scale_B) * rm            [4, 4096]   (ACT tanh + DVE mul)
  - A_t = tanh(scale_A_shard.T)        [4, 512]
  - per i-tile kt: S2T[i128, o512] = matmul(lhsT=Bg[:, kt], rhs=A_t)
    WeffT[:, kt, :] = S2T * wT_tile    (DVE)  -> resident SBUF [128,32,512]
  - main: for nt(8) x kt(32) x ot(4):
      psum[ot] += WeffT[kt, ot*128:+128].T @ xT[kt, nt*512:+512]
    (float32r matmuls: full-rate fp32-storage mode on the PE)
  - psum eviction adds bias (DVE) and DMAs out[o128, n512] tiles.
"""
import sys

if "/opt/trn_rl_repo" not in sys.path:
    sys.path.insert(0, "/opt/trn_rl_repo")

import numpy as np

import concourse.bass as bass  # noqa: F401  (engine types)
import concourse.mybir as mybir
import concourse.tile as tile
from concourse import bacc
from concourse.bass_utils import run_bass_kernel_spmd

# Problem shapes (hardcoded per harness contract)
B, S, I, O, R = 2, 2048, 4096, 4096, 4
N = B * S                  # 4096 rows
NCORES = 8
O_SH = O // NCORES         # 512 out-features per core
P = 128
KT = I // P                # 32 contraction tiles
NT = N // 512              # 8 moving-dim tiles of 512
OT = O_SH // P             # 4 psum partition tiles
KTG = 8                    # kt tiles per x DMA chunk
F32 = mybir.dt.float32
F32R = mybir.dt.float32r


def build_graph():
    nc = bacc.Bacc("TRN2", target_bir_lowering=False, debug=False)

    xT = nc.declare_dram_parameter("xT", [I, N], F32R, isOutput=False)
    wT = nc.declare_dram_parameter("wT", [I, O_SH], F32, isOutput=False)
    scale_a_t = nc.declare_dram_parameter("scale_a_t", [R, O_SH], F32R, isOutput=False)
    scale_b = nc.declare_dram_parameter("scale_b", [R, I], F32R, isOutput=False)
    rm = nc.declare_dram_parameter("rm", [R, 1], F32, isOutput=False)
    bias_t = nc.declare_dram_parameter("bias_t", [P, OT], F32, isOutput=False)
    out = nc.declare_dram_parameter("out", [O_SH, N], F32, isOutput=True)

    xT_r = xT.ap().rearrange("(kt p) n -> p kt n", p=P)     # [128, 32, 4096]
    out_r = out.ap().rearrange("(ot p) n -> p ot n", p=P)   # [128, 4, 4096]
    wT_r = wT.ap().rearrange("(kt p) o -> p kt o", p=P)     # [128, 32, 512]

    with tile.TileContext(nc) as tc:
        with tc.tile_pool(name="const", bufs=1) as const, \
             tc.tile_pool(name="wpool", bufs=3) as wpool, \
             tc.tile_pool(name="xpool", bufs=4) as xpool, \
             tc.tile_pool(name="opool", bufs=2) as opool, \
             tc.tile_pool(name="ps_out", bufs=4, space="PSUM") as ps_out_pool, \
             tc.tile_pool(name="ps_s2t", bufs=2, space="PSUM") as ps_s2t_pool:

            # ---- tiny constants ----
            sb_rm = const.tile([R, 1], F32)
            nc.sync.dma_start(sb_rm[:], rm.ap())
            sb_at = const.tile([R, O_SH], F32R)
            nc.sync.dma_start(sb_at[:], scale_a_t.ap())
            nc.scalar.activation(sb_at[:], sb_at[:], mybir.ActivationFunctionType.Tanh)
            nc.vector.tensor_mul(sb_at[:], sb_at[:], sb_rm[:].to_broadcast((R, O_SH)))

            sb_bg = const.tile([R, I], F32R)
            nc.sync.dma_start(sb_bg[:], scale_b.ap())
            for bc in range(8):
                nc.scalar.activation(
                    sb_bg[:, bc * 512:(bc + 1) * 512],
                    sb_bg[:, bc * 512:(bc + 1) * 512],
                    mybir.ActivationFunctionType.Tanh,
                )

            sb_bias = const.tile([P, OT], F32)
            nc.sync.dma_start(sb_bias[:], bias_t.ap())

            # ---- WeffT = wT * (Bg.T @ (A_t*rm)) built lazily, interleaved
            # into nt0's matmul stream with a small lookahead so the PE never
            # sits idle behind the DVE multiplies.
            weffT = const.tile([P, KT, O_SH], F32R)         # 64 KiB/partition
            WKTG = 4
            wt_tiles = {}

            def emit_wefft_unit(u):
                # unit u covers kt = 2u, 2u+1: two S2T matmuls into a 2-bank
                # psum tile, ONE DVE multiply for both -> halves DVE op/sem
                # overhead on the weffT critical chain.
                ktg = (2 * u) // WKTG
                if (2 * u) % WKTG == 0:
                    wt = wpool.tile([P, WKTG, O_SH], F32, name=f"wt_{ktg}", tag="wt")
                    nc.sync.dma_start(wt[:], wT_r[:, ktg * WKTG:(ktg + 1) * WKTG, :])
                    wt_tiles[ktg] = wt
                ps = ps_s2t_pool.tile([P, 2, O_SH], F32, name=f"ps_s2t_{u}", tag="ps_s2t")
                for j in range(2):
                    kt = 2 * u + j
                    nc.tensor.matmul(
                        ps[:, j, :],
                        sb_bg[:, kt * P:(kt + 1) * P],
                        sb_at[:],
                        start=True, stop=True,
                    )
                wt = wt_tiles[ktg]
                wo = (2 * u) % WKTG
                nc.vector.tensor_mul(
                    weffT[:, 2 * u:2 * u + 2, :], ps[:], wt[:, wo:wo + 2, :])

            LOOKAHEAD = 4   # kt-units of lookahead: 2 units = 4 kt
            for u in range(LOOKAHEAD // 2):
                emit_wefft_unit(u)

            # ---- main matmul: out[o, n] = WeffT.T @ xT + bias ----
            for nt in range(NT):
                x_tiles = []
                for ktg in range(KT // KTG):
                    xt = xpool.tile([P, KTG, 512], F32R, name=f"xt_{nt}_{ktg}", tag="xt")
                    nc.gpsimd.dma_start(
                        xt[:],
                        xT_r[:, ktg * KTG:(ktg + 1) * KTG, nt * 512:(nt + 1) * 512],
                    )
                    x_tiles.append(xt)
                ps_tiles = [
                    ps_out_pool.tile([P, 512], F32, name=f"ps_out_{nt}_{ot}", tag="ps_out")
                    for ot in range(OT)
                ]
                for kt in range(KT):
                    if nt == 0 and kt % 2 == 0 and kt + LOOKAHEAD < KT:
                        emit_wefft_unit((kt + LOOKAHEAD) // 2)
                    xt = x_tiles[kt // KTG]
                    for ot in range(OT):
                        nc.tensor.matmul(
                            ps_tiles[ot][:],
                            weffT[:, kt, ot * P:(ot + 1) * P],
                            xt[:, kt % KTG, :],
                            start=(kt == 0), stop=(kt == KT - 1),
                        )
                o_sb = opool.tile([P, OT, 512], F32, name=f"o_sb_{nt}", tag="o_sb")
                for ot in range(OT):
                    nc.vector.tensor_tensor(
                        o_sb[:, ot, :], ps_tiles[ot][:],
                        sb_bias[:, ot:ot + 1].to_broadcast((P, 512)),
                        mybir.AluOpType.add,
                    )
                nc.scalar.dma_start(
                    out_r[:, :, nt * 512:(nt + 1) * 512],
                    o_sb[:],
                )
    nc.compile()
    return nc


_NC_CACHE = None


def _get_graph():
    global _NC_CACHE
    if _NC_CACHE is None:
        _NC_CACHE = build_graph()
    return _NC_CACHE


def make_in_maps(x, weight, bias, scale_A, scale_B, rank_magnitude):
    x = np.asarray(x, dtype=np.float32)
    weight = np.asarray(weight, dtype=np.float32)
    bias = np.asarray(bias, dtype=np.float32)
    scale_A = np.asarray(scale_A, dtype=np.float32)
    scale_B = np.asarray(scale_B, dtype=np.float32)
    rank_magnitude = np.asarray(rank_magnitude, dtype=np.float32)

    xT = np.ascontiguousarray(x.reshape(N, I).T)            # [I, N]
    in_maps = []
    for c in range(NCORES):
        sl = slice(c * O_SH, (c + 1) * O_SH)
        in_maps.append({
            "xT": xT,
            "wT": np.ascontiguousarray(weight[sl].T),       # [I, O_SH]
            "scale_a_t": np.ascontiguousarray(scale_A[sl].T),  # [R, O_SH]
            "scale_b": scale_B,                              # [R, I]
            "rm": rank_magnitude.reshape(R, 1),
            "bias_t": np.ascontiguousarray(bias[sl].reshape(OT, P).T),  # [128, 4]
        })
    return in_maps


def run(in_maps, trace=False):
    nc = _get_graph()
    return run_bass_kernel_spmd(nc, in_maps, core_ids=list(range(NCORES)), trace=trace)


def assemble(results):
    parts = [results[c]["out"] for c in range(NCORES)]       # each [O_SH, N]
    y_t = np.concatenate(parts, axis=0)                      # [O, N]
    return np.ascontiguousarray(y_t.T).reshape(B, S, O)


def kernel(x, weight, bias, scale_A, scale_B, rank_magnitude):
    in_maps = make_in_maps(x, weight, bias, scale_A, scale_B, rank_magnitude)
    res = run(in_maps, trace=False)
    return assemble(res.results)
